# revision 5
# baseline (speedup 1.0000x reference)
"""Trainium2 Bass kernel for a 2-layer GCN (PyG GCNConv + dense layer).

Computation (matches the jax reference):
    deg[n]  = 1 + sum of incoming edge weights        (self loop weight 1)
    dinv    = deg ** -0.5
    norm_e  = dinv[src] * ew * dinv[dst]              (per real edge)
    agg[n]  = dinv[n]^2 * x[n] + sum_e norm_e * x[src_e]   (propagate FIRST)
    h       = relu(agg @ W1 + b1)
    out     = relu(h @ W2 + b2)

Distribution: nodes (as scatter destinations) are partitioned across the 8
cores; each core gathers the x-rows for its incoming REAL edges straight from
a replicated bf16 x table in HBM (dma_gather; the table is stored
even-nodes-first so both int16-indexed views are N/2 rows), turns each
128-edge chunk into a [edges x nodes] selection matrix S holding the edge
norms (one single-src DVE tensor_scalar against an iota tile, eligible for
the 2x/4x perf modes) and accumulates  msg^T @ S  into PSUM, giving the
aggregation feature-major.  Self-loop contributions dinv^2 * x are
precomputed on the host (dense, feature-major, at tile-slot columns) and
added during the PSUM eviction, so no self-loop rows are ever gathered.
W1/W2 matmuls run feature-major in bf16 with nodes on the moving dimension;
the output is written feature-major and un-transposed on the host.

Known HW constraints baked in: dma_gather is limited to 1024 indices per
single-packet instruction; gathers rotate over 4 SWDGE queues to parallelize
Q7 descriptor generation; per-gather touch matmuls keep S3_LW matmuls at <=2
sync waits; fp32 matmuls run 2 HW passes (hi/lo) so the whole matmul path is
bf16 (1 pass + fast weight load).

Host-side work is limited to graph preprocessing: degree / norm computation
(O(E) scalar ops), edge bucketing by destination tile, and the final
transpose + row un-permutation of the outputs.
"""

import os
import sys

import numpy as np

sys.path.insert(0, "/opt/trn_rl_repo")

P = 128
N_CORES = 8
HALF = 32768          # int16 index limit per gather table view
G_TILES = 4           # node tiles per gather batch (double buffered)

D_IN = 128
D_HID = 512
D_OUT = 128


def _greedy_tiles(cnt_a, cnt_b, n_tiles):
    """Assign local nodes to n_tiles bins of <=P nodes, jointly balancing the
    per-tile A-half and B-half incoming-edge counts (each half's max drives a
    padded chunk count for every tile on every core).
    Returns tile_of[node], pos_in_tile[node]."""
    n = len(cnt_a)
    tot = cnt_a + cnt_b
    order = np.argsort(-tot, kind="stable")
    tile_of = np.empty(n, np.int32)
    pos_in_tile = np.empty(n, np.int32)
    counts = np.zeros(n_tiles, np.int32)
    loadA = np.zeros(n_tiles, np.float64)
    loadB = np.zeros(n_tiles, np.float64)
    tgtA = max(1.0, cnt_a.sum() / n_tiles)
    tgtB = max(1.0, cnt_b.sum() / n_tiles)
    big = np.float64(1e18)
    for node in order:
        score = np.maximum((loadA + cnt_a[node]) / tgtA,
                           (loadB + cnt_b[node]) / tgtB)
        score = np.where(counts < P, score, big)
        t = int(np.argmin(score))
        tile_of[node] = t
        pos_in_tile[node] = counts[t]
        counts[t] += 1
        loadA[t] += cnt_a[node]
        loadB[t] += cnt_b[node]
    return tile_of, pos_in_tile


def _preprocess(x, edge_index, edge_weight):
    """Full-graph preprocessing; returns per-core packed arrays + layout."""
    N = x.shape[0]
    n_per = N // N_CORES
    assert n_per * N_CORES == N

    src = np.asarray(edge_index[0], np.int64)
    dst = np.asarray(edge_index[1], np.int64)
    ew = np.asarray(edge_weight, np.float32)

    deg = np.bincount(dst, weights=ew.astype(np.float64), minlength=N)
    deg = (deg + 1.0).astype(np.float32)          # +1 for the self loop
    dinv = np.where(deg > 0, 1.0 / np.sqrt(deg), 0.0).astype(np.float32)
    norm = (ew * dinv[src] * dinv[dst]).astype(np.float32)

    n_tiles = -(-n_per // P)              # real tiles per core
    n_batches = -(-n_tiles // G_TILES)
    tiles_tot = n_batches * G_TILES       # padded tile count (ghost tiles)

    # Table views are int16-indexed (<=32768 rows each): even nodes first,
    # odd nodes second, so BOTH views are N/2 rows and every core's edge mix
    # is ~50/50 across views.
    interleave = N > HALF
    rows_a = (N + 1) // 2 if interleave else N
    rows_b = N // 2 if interleave else 0

    cores = []
    for c in range(N_CORES):
        lo, hi = c * n_per, (c + 1) * n_per
        m = (dst >= lo) & (dst < hi)
        es = src[m]
        ed = (dst[m] - lo).astype(np.int64)
        en = norm[m]
        if interleave:
            e_half = (es % 2).astype(np.int64)      # odd src -> B view
            e_idx = (es // 2).astype(np.int64)
        else:
            e_half = np.zeros(len(es), np.int64)
            e_idx = es
        cnt_a = np.bincount(ed[e_half == 0], minlength=n_per)
        cnt_b = np.bincount(ed[e_half == 1], minlength=n_per)
        tile_of, pos_in_tile = _greedy_tiles(cnt_a, cnt_b, n_tiles)

        te = tile_of[ed]
        order = np.lexsort((e_idx, e_half, te))
        eidx = e_idx[order]
        ed, en, te, he = ed[order], en[order], te[order], e_half[order]

        seg = te * 2 + he                      # sorted ascending now
        seg_starts = np.searchsorted(seg, np.arange(tiles_tot * 2), side="left")
        rank = np.arange(len(eidx)) - seg_starts[seg]
        lenA = np.bincount(te[he == 0], minlength=tiles_tot)
        lenB = np.bincount(te[he == 1], minlength=tiles_tot)

        cores.append(dict(eidx=eidx, en=en, ed=ed, te=te, he=he, rank=rank,
                          lenA=lenA, lenB=lenB, tile_of=tile_of,
                          pos_in_tile=pos_in_tile, lo=lo, dinv=dinv[lo:hi]))

    K_A = max(1, int(max(-(-core["lenA"].max() // P) for core in cores)))
    if interleave:
        K_B = max(1, int(max(-(-core["lenB"].max() // P) for core in cores)))
    else:
        K_B = 0
    K = K_A + K_B
    n_slots = tiles_tot * K

    per_core = []
    for core in cores:
        gidx = np.zeros(n_slots * P, np.int16)
        mnorm = np.zeros(n_slots * P, np.float32)
        mdst = np.zeros(n_slots * P, np.float32)

        te, he, rank = core["te"], core["he"], core["rank"]
        g = te // G_TILES
        tb = te % G_TILES
        jc = rank // P
        pp = rank % P
        bK = G_TILES * K
        slot = np.where(
            he == 0,
            g * bK + tb * K_A + jc,
            g * bK + G_TILES * K_A + tb * K_B + jc,
        )
        lin = slot * P + pp
        gidx[lin] = core["eidx"].astype(np.int16)
        mnorm[lin] = core["en"]
        mdst[lin] = core["pos_in_tile"][core["ed"]].astype(np.float32)

        # index list wrapped into 16 partitions, replicated to 128
        g16 = gidx.reshape(-1, 16).T.copy()             # [16, n_slots*8]
        g128 = np.tile(g16, (8, 1))                     # [128, n_slots*8]

        # permutation: tile-slot row -> global node id (-1 for ghosts)
        n_per = len(core["tile_of"])
        perm = np.full(tiles_tot * P, -1, np.int64)
        node_rows = core["tile_of"].astype(np.int64) * P + core["pos_in_tile"]
        perm[node_rows] = np.arange(n_per) + core["lo"]

        # self-loop contribution, feature-major at tile-slot columns:
        # xsc[f, tile*P + pos] = x[v, f] * dinv[v]^2
        xsc = np.zeros((D_IN, tiles_tot * P), np.float32)
        vids = np.arange(n_per) + core["lo"]
        xsc[:, node_rows] = (x[vids] * (core["dinv"] ** 2)[:, None]).T

        per_core.append(dict(
            gidx=g128,
            mnorm=mnorm.reshape(n_slots, P).T.copy(),   # [128, n_slots]
            mdst=mdst.reshape(n_slots, P).T.copy(),
            xsc=xsc,
            perm=perm,
        ))

    layout = dict(K_A=K_A, K_B=K_B, K=K, n_slots=n_slots,
                  n_batches=n_batches, tiles_tot=tiles_tot, n_tiles_real=n_tiles,
                  n_rows_A=rows_a, n_rows_B=rows_b)
    return per_core, layout


def _build_program(layout):
    from concourse import bacc, mybir, tile

    f32 = mybir.dt.float32
    bf16 = mybir.dt.bfloat16
    i16 = mybir.dt.int16
    K_A, K_B, K = layout["K_A"], layout["K_B"], layout["K"]
    n_batches = layout["n_batches"]
    n_slots = layout["n_slots"]
    tiles_tot = layout["tiles_tot"]
    N = layout["n_rows_A"] + layout["n_rows_B"]
    out_cols = tiles_tot * P
    bK = G_TILES * K                  # slots per batch
    idx_cols = n_slots * P // 16

    # f32 constants: b1 (4 cols) | b2 (1) | mnorm | mdst | xsc (tiles_tot*P)
    O_B1, O_B2 = 0, 4
    O_MNORM = 5
    O_MDST = O_MNORM + n_slots
    O_XSC = O_MDST + n_slots
    C_COLS = O_XSC + out_cols

    # bf16 constants: w1 (512) | w2r (512) | iota (128)
    O_W1, O_W2 = 0, 512
    O_IOTA = 1024
    C16_COLS = O_IOTA + P

    nc = bacc.Bacc("TRN2", num_swdge_queues=4)
    xtab = nc.declare_dram_parameter("xtab", [N, D_IN], bf16, isOutput=False)
    cdata_d = nc.declare_dram_parameter("cdata", [P, C_COLS], f32, isOutput=False)
    cd16_d = nc.declare_dram_parameter("cd16", [P, C16_COLS], bf16, isOutput=False)
    gidx_d = nc.declare_dram_parameter("gidx", [P, idx_cols], i16, isOutput=False)
    out_d = nc.declare_dram_parameter("out", [P, out_cols], f32, isOutput=True)

    with tile.TileContext(nc) as tc:
        with (
            tc.tile_pool(name="const", bufs=1) as const,
            tc.tile_pool(name="gbuf", bufs=2) as gbuf,
            tc.tile_pool(name="spool", bufs=20) as spool,
            tc.tile_pool(name="aggp", bufs=3) as aggp,
            tc.tile_pool(name="hp", bufs=3) as hp,
            tc.tile_pool(name="outp", bufs=3) as outp,
            tc.tile_pool(name="psa", bufs=2, space="PSUM") as psa,
            tc.tile_pool(name="psh", bufs=2, space="PSUM") as psh,
            tc.tile_pool(name="pso", bufs=2, space="PSUM") as pso,
            tc.tile_pool(name="pst", bufs=2, space="PSUM") as pst,
        ):
            # ---- constants: packed DMAs + the int16 index stream ----
            cdata_s = const.tile([P, C_COLS], f32)
            nc.sync.dma_start(out=cdata_s[:], in_=cdata_d[:])
            cd16_s = const.tile([P, C16_COLS], bf16)
            nc.sync.dma_start(out=cd16_s[:], in_=cd16_d[:])
            gidx_s = const.tile([P, idx_cols], i16)
            nc.sync.dma_start(out=gidx_s[:], in_=gidx_d[:])

            def w1_sl(cc):
                return cd16_s[:, O_W1 + cc * P:O_W1 + (cc + 1) * P]

            def w2_sl(cc):
                return cd16_s[:, O_W2 + cc * P:O_W2 + (cc + 1) * P]

            def b1_sl(cc):
                return cdata_s[:, O_B1 + cc:O_B1 + cc + 1]

            b2_sl = cdata_s[:, O_B2:O_B2 + 1]
            iota_s = cd16_s[:, O_IOTA:O_IOTA + P]
            relu = mybir.ActivationFunctionType.Relu

            GMAX = 8              # slots per gather inst (1024 idx: the
                                  # single-packet SWDGE limit)
            gq = [0]

            def emit_gathers(gb_tile, n_sl, tab_lo, tab_hi, col0):
                for s0 in range(0, n_sl, GMAX):
                    s1 = min(s0 + GMAX, n_sl)
                    ni = (s1 - s0) * P
                    nc.gpsimd.dma_gather(
                        out_ap=gb_tile[:, s0:s1, :],
                        in_ap=xtab[tab_lo:tab_hi, :],
                        idxs_ap=gidx_s[:, col0 + s0 * 8:col0 + s1 * 8],
                        num_idxs=ni, num_idxs_reg=ni,
                        elem_size=D_IN, queue_num=gq[0] % 4,
                        single_packet=True,
                    )
                    gq[0] += 1
                    # touch with a 1-col dummy matmul so downstream matmuls
                    # see the DMA completion through PE engine order (keeps
                    # every S3_LW matmul at <=2 sync waits)
                    ptouch = pst.tile([P, 1], f32, space="PSUM", tag="pt")
                    nc.tensor.matmul(out=ptouch[0:1, :],
                                     lhsT=gb_tile[:, s0, 0:1],
                                     rhs=gb_tile[:, s0, 0:1],
                                     start=True, stop=True)

            n_tiles_real = layout["n_tiles_real"]
            for g in range(n_batches):
                n_rt = max(0, min(G_TILES, n_tiles_real - g * G_TILES))
                col0 = g * (bK * P // 16)
                gbA = gbuf.tile([P, G_TILES * K_A, D_IN], bf16, tag="gbA")
                emit_gathers(gbA, n_rt * K_A, 0, layout["n_rows_A"], col0)
                gbB = None
                if K_B > 0:
                    gbB = gbuf.tile([P, G_TILES * K_B, D_IN], bf16, tag="gbB")
                    emit_gathers(gbB, n_rt * K_B, layout["n_rows_A"], N,
                                 col0 + G_TILES * K_A * 8)

                pagg = psa.tile([P, G_TILES * P], f32, space="PSUM")
                if n_rt < G_TILES:
                    # ghost-tile columns get no matmuls; init them so the
                    # group-wide eviction reads defined data
                    nc.vector.memset(pagg[:, n_rt * P:], 0)
                for tb in range(n_rt):
                    chunks = (
                        [("A", tb * K_A + j) for j in range(K_A)]
                        + [("B", tb * K_B + j) for j in range(K_B)]
                    )
                    for j, (half, sl) in enumerate(chunks):
                        scol = g * bK + (sl if half == "A"
                                         else G_TILES * K_A + sl)
                        src_tile = gbA if half == "A" else gbB
                        S = spool.tile([P, P], bf16, tag="S")
                        nc.vector.tensor_scalar(
                            out=S[:],
                            in0=iota_s,
                            scalar1=cdata_s[:, O_MDST + scol:O_MDST + scol + 1],
                            scalar2=cdata_s[:, O_MNORM + scol:O_MNORM + scol + 1],
                            op0=mybir.AluOpType.is_equal,
                            op1=mybir.AluOpType.mult,
                        )
                        nc.tensor.matmul(
                            out=pagg[:, tb * P:(tb + 1) * P],
                            lhsT=src_tile[:, sl, :],
                            rhs=S[:],
                            start=(j == 0),
                            stop=(j == len(chunks) - 1),
                        )

                # evict + add the host-precomputed self-loop term; bf16 out
                aggT = aggp.tile([P, G_TILES * P], bf16)
                xsc_sl = cdata_s[:, O_XSC + g * G_TILES * P:
                                 O_XSC + (g + 1) * G_TILES * P]
                nc.vector.tensor_tensor(
                    out=aggT[:], in0=pagg[:], in1=xsc_sl,
                    op=mybir.AluOpType.add,
                )

                # layer 1: hT[c] = relu(W1c^T @ aggT + b1c)
                hT = hp.tile([P, 4, G_TILES * P], bf16)
                for cc in range(4):
                    ph = psh.tile([P, G_TILES * P], f32, space="PSUM")
                    nc.tensor.matmul(
                        out=ph[:],
                        lhsT=w1_sl(cc),
                        rhs=aggT[:],
                        start=True, stop=True,
                    )
                    nc.scalar.activation(
                        out=hT[:, cc, :], in_=ph[:], func=relu,
                        bias=b1_sl(cc), scale=1.0,
                    )

                # layer 2: outT = relu(sum_c W2c^T @ hT[c] + b2)
                po = pso.tile([P, G_TILES * P], f32, space="PSUM")
                for cc in range(4):
                    nc.tensor.matmul(
                        out=po[:],
                        lhsT=w2_sl(cc),
                        rhs=hT[:, cc, :],
                        start=(cc == 0), stop=(cc == 3),
                    )
                outT = outp.tile([P, G_TILES * P], f32, tag="outT")
                nc.scalar.activation(
                    out=outT[:], in_=po[:], func=relu,
                    bias=b2_sl, scale=1.0,
                )
                # feature-major output; host un-transposes
                nc.sync.dma_start(
                    out=out_d[:, g * G_TILES * P:(g + 1) * G_TILES * P],
                    in_=outT[:])

    nc.compile()
    return nc


def _install_ntff_hook():
    """The agent image's antenv lacks axon_hooks; fabricate it so trace=True
    can drive NTFF profiling through libaxon_pjrt.so's C ABI."""
    import contextlib
    import ctypes
    import types

    if "antenv.axon_hooks" in sys.modules:
        return
    so_path = "/opt/axon/libaxon_pjrt.so"
    if not os.path.exists(so_path):
        return
    lib = ctypes.CDLL(so_path)
    if not hasattr(lib, "axon_start_nrt_profile"):
        return
    lib.axon_start_nrt_profile.argtypes = [
        ctypes.POINTER(ctypes.c_int64), ctypes.c_size_t]
    lib.axon_start_nrt_profile.restype = ctypes.c_int64
    lib.axon_stop_nrt_profile.argtypes = [ctypes.c_char_p]
    lib.axon_stop_nrt_profile.restype = ctypes.c_int64

    @contextlib.contextmanager
    def _hook(output_dir, device_ids):
        import jax
        jax.devices()
        if device_ids:
            ids = (ctypes.c_int64 * len(device_ids))(*device_ids)
            rc = lib.axon_start_nrt_profile(ids, len(device_ids))
        else:
            rc = lib.axon_start_nrt_profile(None, 0)
        if rc != 0:
            raise RuntimeError(f"axon_start_nrt_profile rc={rc}")
        try:
            yield
        finally:
            n = lib.axon_stop_nrt_profile(str(output_dir).encode())
            print(f"ntff profile: {n} file(s) written to {output_dir}",
                  file=sys.stderr)

    import antenv  # noqa: F401
    mod = types.ModuleType("antenv.axon_hooks")
    mod._hook = _hook
    mod.set_axon_ntff_profile_hook = lambda h: setattr(mod, "_hook", h)
    mod.get_axon_ntff_profile_hook = lambda: mod._hook
    sys.modules["antenv.axon_hooks"] = mod


def _assemble_inputs(x, W1, b1, W2, b2, per_core, layout):
    import ml_dtypes

    iota = np.tile(np.arange(P, dtype=np.float32), (P, 1))
    w2r = W2.reshape(4, P, D_OUT).transpose(1, 0, 2).reshape(P, 4 * D_OUT)
    b1c = b1.reshape(4, P).T
    b2c = b2.reshape(P, 1)

    N = x.shape[0]
    if N > HALF:
        # even nodes first, odd nodes second (matches _preprocess views)
        xt = np.empty_like(x)
        xt[:(N + 1) // 2] = x[0::2]
        xt[(N + 1) // 2:] = x[1::2]
    else:
        xt = x
    xtab_arr = np.ascontiguousarray(xt.astype(ml_dtypes.bfloat16))

    in_maps = []
    for pc in per_core:
        cdata = np.concatenate(
            [b1c, b2c, pc["mnorm"], pc["mdst"], pc["xsc"]],
            axis=1).astype(np.float32)
        cd16 = np.concatenate(
            [W1, w2r, iota], axis=1).astype(ml_dtypes.bfloat16)
        in_maps.append({
            "xtab": xtab_arr,
            "cdata": np.ascontiguousarray(cdata),
            "cd16": np.ascontiguousarray(cd16),
            "gidx": pc["gidx"],
        })
    return in_maps


def _run(nc, in_maps, trace=False):
    if trace:
        try:
            _install_ntff_hook()
        except Exception as e:  # degrade to untraced run
            print(f"ntff hook install failed: {e}", file=sys.stderr)
    from concourse.bass_utils import run_bass_kernel_spmd

    return run_bass_kernel_spmd(
        nc, in_maps, core_ids=list(range(N_CORES)), trace=trace,
    )


def kernel(x, edge_index, edge_weight, W1, b1, W2, b2, _want_trace=False):
    x = np.ascontiguousarray(np.asarray(x, np.float32))
    W1 = np.asarray(W1, np.float32)
    b1 = np.asarray(b1, np.float32)
    W2 = np.asarray(W2, np.float32)
    b2 = np.asarray(b2, np.float32)

    N = x.shape[0]
    per_core, layout = _preprocess(x, edge_index, edge_weight)
    nc = _build_program(layout)

    in_maps = _assemble_inputs(x, W1, b1, W2, b2, per_core, layout)
    res = _run(nc, in_maps, trace=_want_trace)

    out = np.empty((N, D_IN), np.float32)
    for c in range(N_CORES):
        rows = np.ascontiguousarray(res.results[c]["out"].T)
        perm = per_core[c]["perm"]
        valid = perm >= 0
        out[perm[valid]] = rows[valid]

    kernel.last_results = res
    return out


# revision 20
# speedup vs baseline: 1.1008x; 1.1008x over previous
"""Trainium2 Bass kernel for a 2-layer GCN (PyG GCNConv + dense layer).

Computation (matches the jax reference):
    deg[n]  = 1 + sum of incoming edge weights        (self loop weight 1)
    dinv    = deg ** -0.5
    norm_e  = dinv[src] * ew * dinv[dst]              (per real edge)
    agg[n]  = dinv[n]^2 * x[n] + sum_e norm_e * x[src_e]   (propagate FIRST)
    h       = relu(agg @ W1 + b1)
    out     = relu(h @ W2 + b2)

Distribution: nodes (as scatter destinations) are partitioned across the 8
cores; each core gathers the x-rows for its incoming REAL edges straight from
a replicated bf16 x table in HBM (dma_gather; the table is stored
even-nodes-first so both int16-indexed views are N/2 rows), turns each
128-edge chunk into a [edges x nodes] selection matrix S holding the edge
norms (one single-src DVE tensor_scalar against an iota tile, eligible for
the 2x/4x perf modes) and accumulates  msg^T @ S  into PSUM, giving the
aggregation feature-major.  Self-loop contributions dinv^2 * x are
precomputed on the host (dense, feature-major, at tile-slot columns) and
added during the PSUM eviction, so no self-loop rows are ever gathered.
W1/W2 matmuls run feature-major in bf16 with nodes on the moving dimension;
the output is written feature-major and un-transposed on the host.

Known HW constraints baked in: dma_gather is limited to 1024 indices per
single-packet instruction; gathers rotate over 4 SWDGE queues to parallelize
Q7 descriptor generation; per-gather touch matmuls keep S3_LW matmuls at <=2
sync waits; fp32 matmuls run 2 HW passes (hi/lo) so the whole matmul path is
bf16 (1 pass + fast weight load).

Host-side work is limited to graph preprocessing: degree / norm computation
(O(E) scalar ops), edge bucketing by destination tile, and the final
transpose + row un-permutation of the outputs.
"""

import os
import sys

import numpy as np

sys.path.insert(0, "/opt/trn_rl_repo")

P = 128
N_CORES = 8
HALF = 32768          # int16 index limit per gather table view
G_TILES = 4           # node tiles per gather batch (double buffered)

D_IN = 128
D_HID = 512
D_OUT = 128


def _greedy_tiles(cnt_a, cnt_b, n_tiles, cap_a=None, cap_b=None):
    """Assign local nodes to n_tiles bins of <=P nodes, jointly balancing the
    per-tile A-half and B-half incoming-edge counts (each half's max drives a
    padded chunk count for every tile on every core).  Optional per-tile caps
    steer every tile's half-counts under a prescribed chunk signature so the
    cross-core profile stays aligned; caps are best-effort (overflow into the
    least-loaded bin if nothing fits).
    Returns tile_of[node], pos_in_tile[node]."""
    n = len(cnt_a)
    tot = cnt_a + cnt_b
    order = np.argsort(-tot, kind="stable")
    tile_of = np.empty(n, np.int32)
    pos_in_tile = np.empty(n, np.int32)
    counts = np.zeros(n_tiles, np.int32)
    loadA = np.zeros(n_tiles, np.float64)
    loadB = np.zeros(n_tiles, np.float64)
    tgtA = max(1.0, cnt_a.sum() / n_tiles)
    tgtB = max(1.0, cnt_b.sum() / n_tiles)
    big = np.float64(1e18)
    for node in order:
        a, b = cnt_a[node], cnt_b[node]
        score = np.maximum((loadA + a) / tgtA, (loadB + b) / tgtB)
        score = np.where(counts < P, score, big)
        if cap_a is not None:
            feasible = (loadA + a <= cap_a) & (loadB + b <= cap_b) \
                & (counts < P)
            if feasible.any():
                score = np.where(feasible, score, big)
        t = int(np.argmin(score))
        tile_of[node] = t
        pos_in_tile[node] = counts[t]
        counts[t] += 1
        loadA[t] += a
        loadB[t] += b
    return tile_of, pos_in_tile


def _preprocess(x, edge_index, edge_weight):
    """Full-graph preprocessing; returns per-core packed arrays + layout.

    Layout: per core, nodes are greedily binned into n_tiles tiles of <=128.
    Tiles are ranked by total incoming-edge count (desc); rank r maps to
    (batch r % n_batches, position r // n_batches) so batches carry equal
    load.  The slot grid gives each (rank, half) exactly
    max-over-cores ceil(len/128) slots (the shared program needs one shape),
    rows packed tightly from slot-partition 0; trailing rows of the last
    slot are never gathered (indices padded with -1, which SWDGE skips) and
    their S-columns are killed with mnorm=0.
    """
    N = x.shape[0]
    n_per = N // N_CORES
    assert n_per * N_CORES == N

    src = np.asarray(edge_index[0], np.int64)
    dst = np.asarray(edge_index[1], np.int64)
    ew = np.asarray(edge_weight, np.float32)

    deg = np.bincount(dst, weights=ew.astype(np.float64), minlength=N)
    deg = (deg + 1.0).astype(np.float32)          # +1 for the self loop
    dinv = np.where(deg > 0, 1.0 / np.sqrt(deg), 0.0).astype(np.float32)
    norm = (ew * dinv[src] * dinv[dst]).astype(np.float32)

    n_tiles = -(-n_per // P)              # real tiles per core
    n_batches = -(-n_tiles // G_TILES)
    tiles_tot = n_batches * G_TILES       # padded tile count (ghost tiles)

    # Table views are int16-indexed (<=32768 rows each): even nodes first,
    # odd nodes second, so BOTH views are N/2 rows and every core's edge mix
    # is ~50/50 across views.
    interleave = N > HALF
    rows_a = (N + 1) // 2 if interleave else N
    rows_b = N // 2 if interleave else 0

    cores = []
    for c in range(N_CORES):
        lo, hi = c * n_per, (c + 1) * n_per
        m = (dst >= lo) & (dst < hi)
        es = src[m]
        ed = (dst[m] - lo).astype(np.int64)
        en = norm[m]
        if interleave:
            e_half = (es % 2).astype(np.int64)      # odd src -> B view
            e_idx = (es // 2).astype(np.int64)
        else:
            e_half = np.zeros(len(es), np.int64)
            e_idx = es
        cnt_a = np.bincount(ed[e_half == 0], minlength=n_per)
        cnt_b = np.bincount(ed[e_half == 1], minlength=n_per)
        # capped balancing: N_BIG big tiles absorb the overflow so every
        # other tile's halves stay under SMALL_CAP (one chunk fewer), and
        # the cross-core chunk profile aligns by construction
        N_BIG = 12
        SMALL_CAP = 740        # guard below 6*P so tiny cap overflows stay
        BIG_CAP = 890          # within the 6-chunk budget

        cap_a = np.full(n_tiles, SMALL_CAP, np.float64)
        cap_b = np.full(n_tiles, SMALL_CAP, np.float64)
        cap_a[:N_BIG] = BIG_CAP
        cap_b[:N_BIG] = BIG_CAP
        tile_of, pos_in_tile = _greedy_tiles(cnt_a, cnt_b, n_tiles,
                                             cap_a, cap_b)

        te = tile_of[ed]
        lenA = np.bincount(te[e_half == 0], minlength=n_tiles)
        lenB = np.bincount(te[e_half == 1], minlength=n_tiles)
        # rank big tiles (bins 0..N_BIG-1) first, then small, each group by
        # total load desc, so the per-rank cross-core max stays tight
        load = (lenA + lenB).astype(np.float64)
        grp = np.where(np.arange(n_tiles) < N_BIG, 0, 1)
        order = np.lexsort((-load, grp))
        rank_of_tile = np.empty(n_tiles, np.int64)
        rank_of_tile[order] = np.arange(n_tiles)

        cores.append(dict(es=es, ed=ed, en=en, te=te, he=e_half, eidx=e_idx,
                          lenA=lenA, lenB=lenB, tile_of=tile_of,
                          pos_in_tile=pos_in_tile, lo=lo, dinv=dinv[lo:hi],
                          rank_of_tile=rank_of_tile))

    # shared slot profile: per rank, per half, max over cores of ceil(len/P);
    # niA/niB: shared gather index counts (max actual len, rounded to 16)
    profA = np.zeros(tiles_tot, np.int64)
    profB = np.zeros(tiles_tot, np.int64)
    maxLenA = np.zeros(tiles_tot, np.int64)
    maxLenB = np.zeros(tiles_tot, np.int64)
    for core in cores:
        inv = np.argsort(core["rank_of_tile"])        # rank -> tile id
        profA[:n_tiles] = np.maximum(profA[:n_tiles],
                                     -(-core["lenA"][inv] // P))
        profB[:n_tiles] = np.maximum(profB[:n_tiles],
                                     -(-core["lenB"][inv] // P))
        maxLenA[:n_tiles] = np.maximum(maxLenA[:n_tiles], core["lenA"][inv])
        maxLenB[:n_tiles] = np.maximum(maxLenB[:n_tiles], core["lenB"][inv])
    niA = np.minimum(-(-maxLenA // 16) * 16, profA * P)
    niB = np.minimum(-(-maxLenB // 16) * 16, profB * P)

    # rank r -> (batch, pos); slot base of each (rank, half) in a
    # batch-major contiguous layout
    rank_of = lambda b, i: i * n_batches + b          # noqa: E731
    slot_baseA = np.zeros(tiles_tot, np.int64)
    slot_baseB = np.zeros(tiles_tot, np.int64)
    batch_base = np.zeros(n_batches + 1, np.int64)
    cur = 0
    for b in range(n_batches):
        batch_base[b] = cur
        for i in range(G_TILES):
            r = rank_of(b, i)
            if r >= n_tiles:
                continue
            slot_baseA[r] = cur
            cur += profA[r]
            slot_baseB[r] = cur
            cur += profB[r]
    n_slots = cur
    batch_base[n_batches] = cur
    max_batch_slots = int(np.max(np.diff(batch_base)))

    per_core = []
    for cidx, core in enumerate(cores):
        # order edges by (rank of tile, half, idx); rows pack tightly per
        # (rank, half) starting at its slot base
        rk = core["rank_of_tile"][core["te"]]
        order = np.lexsort((core["eidx"], core["he"], rk))
        eidx = core["eidx"][order]
        ed = core["ed"][order]
        en = core["en"][order]
        rk = rk[order]
        he = core["he"][order]

        seg = rk * 2 + he
        seg_starts = np.searchsorted(seg, np.arange(n_tiles * 2), side="left")
        erank = np.arange(len(eidx)) - seg_starts[seg]
        base = np.where(he == 0, slot_baseA[rk], slot_baseB[rk])
        slot = base + erank // P
        pp = erank % P
        lin = slot * P + pp

        mnorm = np.zeros(n_slots * P, np.float32)
        mdst = np.zeros(n_slots * P, np.float32)
        gidx = np.zeros(n_slots * P, np.int16)   # pad idx 0 = valid row 0
        gidx[lin] = eidx.astype(np.int16)
        mnorm[lin] = en
        mdst[lin] = core["pos_in_tile"][ed].astype(np.float32)

        # index list wrapped into 16 partitions, replicated to 128
        g16 = gidx.reshape(-1, 16).T.copy()             # [16, n_slots*8]
        g128 = np.tile(g16, (8, 1))                     # [128, n_slots*8]

        # permutation: output-column row -> global node id (-1 for ghosts);
        # tile (b, i) owns output columns (b*G_TILES+i)*P ... +P
        n_per_c = len(core["tile_of"])
        rk_t = core["rank_of_tile"][core["tile_of"]]
        b_t = rk_t % n_batches
        i_t = rk_t // n_batches
        node_rows = (b_t * G_TILES + i_t) * P + core["pos_in_tile"]
        perm = np.full(tiles_tot * P, -1, np.int64)
        perm[node_rows] = np.arange(n_per_c) + core["lo"]

        # self-loop contribution, feature-major at output columns:
        # xsc[f, col] = x[v, f] * dinv[v]^2
        xsc = np.zeros((D_IN, tiles_tot * P), np.float32)
        vids = np.arange(n_per_c) + core["lo"]
        xsc[:, node_rows] = (x[vids] * (core["dinv"] ** 2)[:, None]).T

        per_core.append(dict(
            gidx=g128,
            mnorm=mnorm.reshape(n_slots, P).T.copy(),   # [128, n_slots]
            mdst=mdst.reshape(n_slots, P).T.copy(),
            xsc=xsc,
            perm=perm,
        ))

    layout = dict(n_slots=n_slots, n_batches=n_batches, tiles_tot=tiles_tot,
                  n_tiles_real=n_tiles, n_rows_A=rows_a, n_rows_B=rows_b,
                  profA=profA, profB=profB, niA=niA, niB=niB,
                  slot_baseA=slot_baseA, slot_baseB=slot_baseB,
                  batch_base=batch_base, max_batch_slots=max_batch_slots)
    return per_core, layout


def _build_program(layout):
    from concourse import bacc, mybir, tile

    f32 = mybir.dt.float32
    bf16 = mybir.dt.bfloat16
    i16 = mybir.dt.int16
    n_batches = layout["n_batches"]
    n_slots = layout["n_slots"]
    tiles_tot = layout["tiles_tot"]
    n_tiles_real = layout["n_tiles_real"]
    profA, profB = layout["profA"], layout["profB"]
    slot_baseA, slot_baseB = layout["slot_baseA"], layout["slot_baseB"]
    batch_base = layout["batch_base"]
    max_batch_slots = layout["max_batch_slots"]
    N = layout["n_rows_A"] + layout["n_rows_B"]
    out_cols = tiles_tot * P
    idx_cols = n_slots * P // 16

    # f32 constants: b1 (4) | b2 (1) | iota (128) | mnorm | mdst | negmdst
    #                | negmnorm | xsc
    O_B1, O_B2 = 0, 4
    O_IOTA = 5
    O_MNORM = O_IOTA + P
    O_MDST = O_MNORM + n_slots
    O_NEGMDST = O_MDST + n_slots
    O_NEGMNORM = O_NEGMDST + n_slots
    O_XSC = O_NEGMNORM + n_slots
    C_COLS = O_XSC + out_cols

    # bf16 constants: w1 (512) | w2r (512)
    O_W1, O_W2 = 0, 512
    C16_COLS = 1024

    nc = bacc.Bacc("TRN2", num_swdge_queues=4)
    xtab = nc.declare_dram_parameter("xtab", [N, D_IN], bf16, isOutput=False)
    cdata_d = nc.declare_dram_parameter("cdata", [P, C_COLS], f32, isOutput=False)
    cd16_d = nc.declare_dram_parameter("cd16", [P, C16_COLS], bf16, isOutput=False)
    gidx_d = nc.declare_dram_parameter("gidx", [P, idx_cols], i16, isOutput=False)
    out_d = nc.declare_dram_parameter("out", [P, out_cols], f32, isOutput=True)

    with tile.TileContext(nc) as tc:
        with (
            tc.tile_pool(name="const", bufs=1) as const,
            tc.tile_pool(name="gbuf", bufs=2) as gbuf,
            tc.tile_pool(name="spool", bufs=20) as spool,
            tc.tile_pool(name="aggp", bufs=3) as aggp,
            tc.tile_pool(name="hp", bufs=3) as hp,
            tc.tile_pool(name="outp", bufs=3) as outp,
            tc.tile_pool(name="psa", bufs=2, space="PSUM") as psa,
            tc.tile_pool(name="psh", bufs=2, space="PSUM") as psh,
            tc.tile_pool(name="pso", bufs=2, space="PSUM") as pso,
            tc.tile_pool(name="pst", bufs=2, space="PSUM") as pst,
        ):
            # ---- constants: packed DMAs + the int16 index stream ----
            cdata_s = const.tile([P, C_COLS], f32)
            nc.sync.dma_start(out=cdata_s[:], in_=cdata_d[:])
            cd16_s = const.tile([P, C16_COLS], bf16)
            nc.sync.dma_start(out=cd16_s[:], in_=cd16_d[:])
            gidx_s = const.tile([P, idx_cols], i16)
            nc.sync.dma_start(out=gidx_s[:], in_=gidx_d[:])

            def w1_sl(cc):
                return cd16_s[:, O_W1 + cc * P:O_W1 + (cc + 1) * P]

            def w2_sl(cc):
                return cd16_s[:, O_W2 + cc * P:O_W2 + (cc + 1) * P]

            def b1_sl(cc):
                return cdata_s[:, O_B1 + cc:O_B1 + cc + 1]

            b2_sl = cdata_s[:, O_B2:O_B2 + 1]
            iota_s = cdata_s[:, O_IOTA:O_IOTA + P]
            relu = mybir.ActivationFunctionType.Relu

            IMAX = 1024           # idx per gather inst (single-packet SWDGE
                                  # limit)
            gq = [0]

            def emit_gathers(gb_tile, sl_base, n_idx, tab_lo, tab_hi, gcol):
                # gcol: idx-stream column of this gb tile's slot 0; gathers
                # n_idx rows (multiple of 16) tightly from slot sl_base on
                if n_idx == 0:
                    return
                for i0 in range(0, n_idx, IMAX):
                    ni = min(IMAX, n_idx - i0)
                    s0 = sl_base + i0 // P
                    s1 = sl_base + -(-(i0 + ni) // P)
                    c0 = gcol + sl_base * 8 + i0 // 16
                    nc.gpsimd.dma_gather(
                        out_ap=gb_tile[:, s0:s1, :],
                        in_ap=xtab[tab_lo:tab_hi, :],
                        idxs_ap=gidx_s[:, c0:c0 + ni // 16],
                        num_idxs=ni, num_idxs_reg=ni,
                        elem_size=D_IN, queue_num=gq[0] % 4,
                        single_packet=True,
                    )
                    gq[0] += 1
                    # touch with a 1-col dummy matmul so downstream matmuls
                    # see the DMA completion through PE engine order (keeps
                    # every S3_LW matmul at <=2 sync waits)
                    ptouch = pst.tile([P, 1], f32, space="PSUM", tag="pt")
                    nc.tensor.matmul(out=ptouch[0:1, :],
                                     lhsT=gb_tile[:, s0, 0:1],
                                     rhs=gb_tile[:, s0, 0:1],
                                     start=True, stop=True)

            niA, niB = layout["niA"], layout["niB"]
            for g in range(n_batches):
                sb = int(batch_base[g])
                gb = gbuf.tile([P, max_batch_slots, D_IN], bf16, tag="gb")
                if g < 2:
                    # initialize the two gather buffers once: rows that no
                    # gather ever writes must still hold finite bf16 (their
                    # S columns are 0, but 0 * NaN would poison the PSUM)
                    nc.vector.memset(gb[:], 0)
                tiles = []
                for i in range(G_TILES):
                    r = i * n_batches + g
                    if r >= n_tiles_real:
                        continue
                    kA, kB = int(profA[r]), int(profB[r])
                    bA = int(slot_baseA[r]) - sb
                    bB = int(slot_baseB[r]) - sb
                    emit_gathers(gb, bA, int(niA[r]), 0, layout["n_rows_A"],
                                 sb * 8)
                    if kB:
                        emit_gathers(gb, bB, int(niB[r]), layout["n_rows_A"],
                                     N, sb * 8)
                    tiles.append((i, kA, kB, bA, bB))

                pagg = psa.tile([P, G_TILES * P], f32, space="PSUM")
                live = {i for i, *_ in tiles}
                for i in range(G_TILES):
                    if i not in live:
                        # ghost-tile columns get no matmuls; init them so
                        # the group-wide eviction reads defined data
                        nc.vector.memset(pagg[:, i * P:(i + 1) * P], 0)
                for i, kA, kB, bA, bB in tiles:
                    chunks = ([bA + j for j in range(kA)]
                              + [bB + j for j in range(kB)])
                    for j, sl in enumerate(chunks):
                        scol = sb + sl
                        S = spool.tile([P, P], bf16, tag="S")
                        nc.vector.scalar_tensor_tensor(
                            out=S[:],
                            in0=iota_s,
                            scalar=cdata_s[:, O_MDST + scol:O_MDST + scol + 1],
                            in1=cdata_s[:, O_MNORM + scol:O_MNORM + scol + 1]
                            .to_broadcast([P, P]),
                            op0=mybir.AluOpType.is_equal,
                            op1=mybir.AluOpType.mult,
                        )
                        nc.tensor.matmul(
                            out=pagg[:, i * P:(i + 1) * P],
                            lhsT=gb[:, sl, :],
                            rhs=S[:],
                            start=(j == 0),
                            stop=(j == len(chunks) - 1),
                        )

                # evict + add the host-precomputed self-loop term; bf16 out
                aggT = aggp.tile([P, G_TILES * P], bf16)
                xsc_sl = cdata_s[:, O_XSC + g * G_TILES * P:
                                 O_XSC + (g + 1) * G_TILES * P]
                nc.vector.tensor_tensor(
                    out=aggT[:], in0=pagg[:], in1=xsc_sl,
                    op=mybir.AluOpType.add,
                )

                # layer 1: hT[c] = relu(W1c^T @ aggT + b1c)
                hT = hp.tile([P, 4, G_TILES * P], bf16)
                for cc in range(4):
                    ph = psh.tile([P, G_TILES * P], f32, space="PSUM")
                    nc.tensor.matmul(
                        out=ph[:],
                        lhsT=w1_sl(cc),
                        rhs=aggT[:],
                        start=True, stop=True,
                    )
                    nc.scalar.activation(
                        out=hT[:, cc, :], in_=ph[:], func=relu,
                        bias=b1_sl(cc), scale=1.0,
                    )

                # layer 2: outT = relu(sum_c W2c^T @ hT[c] + b2)
                po = pso.tile([P, G_TILES * P], f32, space="PSUM")
                for cc in range(4):
                    nc.tensor.matmul(
                        out=po[:],
                        lhsT=w2_sl(cc),
                        rhs=hT[:, cc, :],
                        start=(cc == 0), stop=(cc == 3),
                    )
                outT = outp.tile([P, G_TILES * P], f32, tag="outT")
                nc.scalar.activation(
                    out=outT[:], in_=po[:], func=relu,
                    bias=b2_sl, scale=1.0,
                )
                # feature-major output; host un-transposes
                nc.sync.dma_start(
                    out=out_d[:, g * G_TILES * P:(g + 1) * G_TILES * P],
                    in_=outT[:])

    nc.compile()
    return nc


def _install_ntff_hook():
    """The agent image's antenv lacks axon_hooks; fabricate it so trace=True
    can drive NTFF profiling through libaxon_pjrt.so's C ABI."""
    import contextlib
    import ctypes
    import types

    if "antenv.axon_hooks" in sys.modules:
        return
    so_path = "/opt/axon/libaxon_pjrt.so"
    if not os.path.exists(so_path):
        return
    lib = ctypes.CDLL(so_path)
    if not hasattr(lib, "axon_start_nrt_profile"):
        return
    lib.axon_start_nrt_profile.argtypes = [
        ctypes.POINTER(ctypes.c_int64), ctypes.c_size_t]
    lib.axon_start_nrt_profile.restype = ctypes.c_int64
    lib.axon_stop_nrt_profile.argtypes = [ctypes.c_char_p]
    lib.axon_stop_nrt_profile.restype = ctypes.c_int64

    @contextlib.contextmanager
    def _hook(output_dir, device_ids):
        import jax
        jax.devices()
        if device_ids:
            ids = (ctypes.c_int64 * len(device_ids))(*device_ids)
            rc = lib.axon_start_nrt_profile(ids, len(device_ids))
        else:
            rc = lib.axon_start_nrt_profile(None, 0)
        if rc != 0:
            raise RuntimeError(f"axon_start_nrt_profile rc={rc}")
        try:
            yield
        finally:
            n = lib.axon_stop_nrt_profile(str(output_dir).encode())
            print(f"ntff profile: {n} file(s) written to {output_dir}",
                  file=sys.stderr)

    import antenv  # noqa: F401
    mod = types.ModuleType("antenv.axon_hooks")
    mod._hook = _hook
    mod.set_axon_ntff_profile_hook = lambda h: setattr(mod, "_hook", h)
    mod.get_axon_ntff_profile_hook = lambda: mod._hook
    sys.modules["antenv.axon_hooks"] = mod


def _assemble_inputs(x, W1, b1, W2, b2, per_core, layout):
    import ml_dtypes

    iota = np.tile(np.arange(P, dtype=np.float32), (P, 1))
    w2r = W2.reshape(4, P, D_OUT).transpose(1, 0, 2).reshape(P, 4 * D_OUT)
    b1c = b1.reshape(4, P).T
    b2c = b2.reshape(P, 1)

    N = x.shape[0]
    if N > HALF:
        # even nodes first, odd nodes second (matches _preprocess views)
        xt = np.empty_like(x)
        xt[:(N + 1) // 2] = x[0::2]
        xt[(N + 1) // 2:] = x[1::2]
    else:
        xt = x
    xtab_arr = np.ascontiguousarray(xt.astype(ml_dtypes.bfloat16))

    in_maps = []
    for pc in per_core:
        cdata = np.concatenate(
            [b1c, b2c, iota, pc["mnorm"], pc["mdst"], pc["xsc"]],
            axis=1).astype(np.float32)
        cd16 = np.concatenate(
            [W1, w2r], axis=1).astype(ml_dtypes.bfloat16)
        in_maps.append({
            "xtab": xtab_arr,
            "cdata": np.ascontiguousarray(cdata),
            "cd16": np.ascontiguousarray(cd16),
            "gidx": pc["gidx"],
        })
    return in_maps


def _run(nc, in_maps, trace=False):
    if trace:
        try:
            _install_ntff_hook()
        except Exception as e:  # degrade to untraced run
            print(f"ntff hook install failed: {e}", file=sys.stderr)
    from concourse.bass_utils import run_bass_kernel_spmd

    return run_bass_kernel_spmd(
        nc, in_maps, core_ids=list(range(N_CORES)), trace=trace,
    )


def kernel(x, edge_index, edge_weight, W1, b1, W2, b2, _want_trace=False):
    x = np.ascontiguousarray(np.asarray(x, np.float32))
    W1 = np.asarray(W1, np.float32)
    b1 = np.asarray(b1, np.float32)
    W2 = np.asarray(W2, np.float32)
    b2 = np.asarray(b2, np.float32)

    N = x.shape[0]
    per_core, layout = _preprocess(x, edge_index, edge_weight)
    nc = _build_program(layout)

    in_maps = _assemble_inputs(x, W1, b1, W2, b2, per_core, layout)
    res = _run(nc, in_maps, trace=_want_trace)

    out = np.empty((N, D_IN), np.float32)
    for c in range(N_CORES):
        rows = np.ascontiguousarray(res.results[c]["out"].T)
        perm = per_core[c]["perm"]
        valid = perm >= 0
        out[perm[valid]] = rows[valid]

    kernel.last_results = res
    return out
